# revision 6
# baseline (speedup 1.0000x reference)
"""C-index kernel v3 for Trainium2, 8 NeuronCores — histogram scheme.

Math (bf16-rounded world; error vs fp32 reference ~1e-3 << 2e-2 gate)
----
y, yh rounded to bf16; both rank-bucketed into K=128 equal buckets
(exactly 128 samples each): p(.) for y, q(.) for yh.  Over the
rectangle (i in [N]) x (j in events E), with
    [y_i>=y_j]  = [p_i>p_j] + [p_i==p_j][y_i>=y_j]
    [yh_i>=yh_j] = [q_i>q_j] + [q_i==q_j][yh_i>=yh_j]
the product expands into four disjoint terms:
    S1 = T_bb + T_byh + T_yb + T_yy
    T_bb  = sum [p_i>p_j][q_i>q_j]                  (histogram product)
    T_byh = sum [q_i==q_j][p_i>p_j][yh_i>=yh_j]     (same-yh-bucket pass)
    T_yb  = sum [p_i==p_j][y_i>=y_j][q_i>q_j]       (same-y-bucket pass)
    T_yy  = sum [p_i==p_j][q_i==q_j][y_i>=y_j][yh_i>=yh_j]
    S2 = T2_b + T2_y;  T2_b = sum [p_i>p_j] (1-D histograms, host),
    T2_y = sum [p_i==p_j][y_i>=y_j] (same-y-bucket pass)
    c = S1 - ns, t = S2 - ns, out = fp32(c)/fp32(t)

Device (i sharded 8 ways = 2048 rows = 16 one-hot tiles; 16 buckets of
each kind per core):
  - Hi = sum_i onehot_p(i) x onehot_q(i): 8 fp8 DoubleRow matmuls into
    PSUM [128, 128]; T_bb = fused mult+accum against host-built
    W[r,s] = #events with p_j<r and q_j<s.
  - bucket passes: packed [128 members x W events] tiles, bf16
    tensor_tensor compares + fused scalar_tensor_tensor accumulates,
    all on DVE (GpSimd is banned: concurrent Pool+DVE ops slow each
    other ~25x on TRN2, measured).
Host: bf16 rounding, argsort bucketing, event histogram + suffix,
packing; final fp64 algebra over [128]-vector accumulators.
"""

import math
import os
import sys

import numpy as np
import ml_dtypes

for _p in ("/opt/trn_rl_repo", "/root/.axon_site", "/root/.axon_site/_ro/trn_rl_repo"):
    if os.path.isdir(_p) and _p not in sys.path:
        sys.path.append(_p)

import concourse.bacc as bacc
import concourse.mybir as mybir
from concourse import bass_utils
from concourse import tile

N = 16384
P = 128
NCORES = 8
TPC = N // NCORES          # 2048 i per core
NT = TPC // P              # 16 i-tiles per core
K = 128                    # buckets in each dim (N/K = 128 exactly)
NB = K // NCORES           # 16 buckets of each kind per core
BIGPAD = ml_dtypes.bfloat16(1e30)
IDXPAD = ml_dtypes.bfloat16(300.0)   # bucket-index pad (> any k)

FP32 = mybir.dt.float32
BF16 = mybir.dt.bfloat16
FP8 = mybir.dt.float8e4
Alu = mybir.AluOpType
ActF = mybir.ActivationFunctionType
DR = mybir.MatmulPerfMode.DoubleRow


def build_bass_full(sbh, sby):
    """sbh/sby: total packed widths of the yh-/y-bucket passes."""
    nc = bacc.Bacc(debug=False, num_devices=NCORES)

    uu = nc.dram_tensor("uu", [P, 2 * NT * K], FP8, kind="ExternalInput")
    wbb = nc.dram_tensor("wbb", [P, K], FP32, kind="ExternalInput")
    harr = nc.dram_tensor("harr", [P, 4 * sbh], BF16, kind="ExternalInput")
    yarr = nc.dram_tensor("yarr", [P, 4 * sby], BF16, kind="ExternalInput")
    o_acc = nc.dram_tensor("o_acc", [P, 4], FP32, kind="ExternalOutput")

    with tile.TileContext(nc) as tc:
        with (
            tc.tile_pool(name="const", bufs=1) as cpool,
            tc.tile_pool(name="scrap", bufs=1) as spool,
            tc.tile_pool(name="psum", bufs=1, space="PSUM") as ppool,
        ):
            # ---- inputs: per-array sub-DMAs in consumption order ----
            t_h = cpool.tile([P, 4, sbh], BF16, name="t_h")
            t_y = cpool.tile([P, 4, sby], BF16, name="t_y")
            qengs = [nc.sync, nc.scalar]
            for xi in range(4):
                qengs[xi % 2].dma_start(
                    out=t_h[:, xi, :], in_=harr[:, xi * sbh:(xi + 1) * sbh])
            for xi in range(4):
                qengs[xi % 2].dma_start(
                    out=t_y[:, xi, :], in_=yarr[:, xi * sby:(xi + 1) * sby])
            t_uu = cpool.tile([P, 2, NT, K], FP8, name="t_uu")
            nc.sync.dma_start(out=t_uu[:, :, :, :], in_=uu[:, :])
            t_wbb = cpool.tile([P, K], FP32, name="t_wbb")
            nc.scalar.dma_start(out=t_wbb[:, :], in_=wbb[:, :])

            accg = cpool.tile([P, 4], FP32, name="accg")
            accs = [accg[:, x:x + 1] for x in range(4)]

            # ---- Hi histogram + T_bb ----
            hi = ppool.tile([P, K], FP32, name="hi")
            for m in range(NT // 2):
                nc.tensor.matmul(
                    hi[:, :],
                    t_uu[:, 0, 2 * m:2 * m + 2, :],
                    t_uu[:, 1, 2 * m:2 * m + 2, :],
                    start=(m == 0), stop=(m == NT // 2 - 1),
                    perf_mode=DR)
            # ---- same-yh-bucket pass: T_byh ----
            # one merged is_le computes [cp | c4h] ([p_j+1 | yh_j] vs
            # [p_i | yh_i]); cp uses integer-id shift [pj<pi]=[pj+1<=pi]
            ch = spool.tile([P, 2, sbh], BF16, name="ch")
            nc.vector.tensor_tensor(
                out=ch[:, :, :], in0=t_h[:, 0:2, :], in1=t_h[:, 2:4, :],
                op=Alu.is_le)
            m1 = spool.tile([P, sbh], BF16, name="m1")
            nc.vector.scalar_tensor_tensor(
                out=m1[:, :], in0=ch[:, 0, :], scalar=1.0, in1=ch[:, 1, :],
                op0=Alu.mult, op1=Alu.mult, accum_out=accs[1])

            # ---- same-y-bucket pass ----
            # within a y-bucket, [q>]+[q=][yh>=] == [yh>=], so the whole
            # same-y-bucket contribution is sum [y>=][yh>=].  One merged
            # is_le computes both [c1 | c4y] (arrays laid out as
            # [y_yj | y_hj] vs [y_yi | y_hi]).
            cc = spool.tile([P, 2, sby], BF16, name="cc")
            nc.vector.tensor_tensor(
                out=cc[:, :, :], in0=t_y[:, 0:2, :], in1=t_y[:, 2:4, :],
                op=Alu.is_le)
            t2y = spool.tile([P, sby], BF16, name="t2y")
            nc.scalar.activation(
                out=t2y[:, :], in_=cc[:, 0, :], func=ActF.Copy,
                bias=0.0, scale=1.0, accum_out=accs[3])
            m4 = spool.tile([P, sby], BF16, name="m4")
            nc.vector.scalar_tensor_tensor(
                out=m4[:, :], in0=cc[:, 1, :], scalar=1.0, in1=cc[:, 0, :],
                op0=Alu.mult, op1=Alu.mult, accum_out=accs[2])

            tbb_out = spool.tile([P, K], FP32, name="tbb_out")
            nc.vector.scalar_tensor_tensor(
                out=tbb_out[:, :], in0=hi[:, :], scalar=1.0,
                in1=t_wbb[:, :], op0=Alu.mult, op1=Alu.mult,
                accum_out=accs[0])

            nc.sync.dma_start(out=o_acc[:, :], in_=accg[:, :])

    nc.compile()
    return nc


_NC_CACHE = {}


def _get_nc(sbh, sby):
    key = (sbh, sby)
    if key not in _NC_CACHE:
        _NC_CACHE[key] = build_bass_full(sbh, sby)
    return _NC_CACHE[key]


def _rank_buckets(v32):
    order = np.argsort(v32, kind="stable")
    r = np.empty(N, np.int64)
    r[order] = np.arange(N)
    return (r * K) // N, order


def _prep(y, yh, status):
    y16 = np.asarray(y, np.float32).astype(ml_dtypes.bfloat16)
    yh16 = np.asarray(yh, np.float32).astype(ml_dtypes.bfloat16)
    st = np.asarray(status)
    ev = np.nonzero(st == 1)[0]
    ns = len(ev)
    p, order_y = _rank_buckets(y16.astype(np.float32))
    q, order_h = _rank_buckets(yh16.astype(np.float32))
    pj, qj = p[ev], q[ev]

    # W[r,s] = #events with p_j < r and q_j < s (strict 2-D prefix)
    hj2 = np.zeros((K, K))
    np.add.at(hj2, (pj, qj), 1.0)
    pref = hj2.cumsum(0).cumsum(1)
    w_strict = np.zeros((K, K), np.float32)
    w_strict[1:, 1:] = pref[:-1, :-1]

    mem_y = order_y.reshape(K, P)
    mem_h = order_h.reshape(K, P)
    ev_by_p = [ev[pj == b] for b in range(K)]
    ev_by_q = [ev[qj == b] for b in range(K)]

    def layout(evb):
        offs = []
        tot = 0
        for c in range(NCORES):
            o = [0]
            for bi in range(NB):
                w = int(math.ceil(max(1, len(evb[c * NB + bi])) / 8)) * 8
                o.append(o[-1] + w)
            offs.append(o)
            tot = max(tot, o[-1])
        return offs, tot

    offs_y, sby = layout(ev_by_p)
    offs_h, sbh = layout(ev_by_q)

    return dict(y16=y16, yh16=yh16, ev=ev, ns=ns, p=p, q=q, pj=pj, qj=qj,
                w_strict=w_strict, mem_y=mem_y, mem_h=mem_h,
                ev_by_p=ev_by_p, ev_by_q=ev_by_q,
                offs_y=offs_y, sby=sby, offs_h=offs_h, sbh=sbh)


def make_in_maps(pp):
    y16, yh16, p, q = pp["y16"], pp["yh16"], pp["p"], pp["q"]
    b16 = ml_dtypes.bfloat16
    p16 = p.astype(b16)
    q16 = q.astype(b16)
    in_maps = []
    for c in range(NCORES):
        i0 = c * TPC
        idx = np.arange(i0, i0 + TPC).reshape(NT, P)
        u_y = np.zeros((P, NT, K), ml_dtypes.float8_e4m3)
        u_h = np.zeros((P, NT, K), ml_dtypes.float8_e4m3)
        for t in range(NT):
            u_y[np.arange(P), t, p[idx[t]]] = 1.0
            u_h[np.arange(P), t, q[idx[t]]] = 1.0

        def pack(mem, evb, offs, tot, jvals, ivals, jpad):
            """[P, tot]: concatenated per-bucket blocks (variable width)."""
            aj = np.full((P, tot), jpad, b16)
            ai = np.zeros((P, tot), b16)
            for bi in range(NB):
                b = c * NB + bi
                e = evb[b]
                o0, o1 = offs[bi], offs[bi + 1]
                ai[:, o0:o1] = ivals[mem[b]][:, None]
                aj[:, o0:o0 + len(e)] = jvals[e][None, :]
            return np.ascontiguousarray(aj), np.ascontiguousarray(ai)

        offs_h, sbh = pp["offs_h"][c], pp["sbh"]
        offs_y, sby = pp["offs_y"][c], pp["sby"]
        h_hj, h_hi = pack(pp["mem_h"], pp["ev_by_q"], offs_h, sbh,
                          yh16, yh16, BIGPAD)
        # [p_j < p_i] == [p_j + 1 <= p_i] for integer bucket ids
        p16p1 = (p.astype(np.float32) + 1.0).astype(b16)
        h_pj, h_pi = pack(pp["mem_h"], pp["ev_by_q"], offs_h, sbh,
                          p16p1, p16, IDXPAD)
        y_yj, y_yi = pack(pp["mem_y"], pp["ev_by_p"], offs_y, sby,
                          y16, y16, BIGPAD)
        y_hj, y_hi = pack(pp["mem_y"], pp["ev_by_p"], offs_y, sby,
                          yh16, yh16, BIGPAD)

        # wbb row r aligns with PSUM partition r (p-bucket)
        in_maps.append({
            "uu": np.ascontiguousarray(
                np.stack([u_y, u_h], axis=1).reshape(P, 2 * NT * K)),
            "wbb": np.ascontiguousarray(pp["w_strict"]),
            "harr": np.ascontiguousarray(
                np.concatenate([h_pj, h_hj, h_pi, h_hi], axis=1)),
            "yarr": np.ascontiguousarray(
                np.concatenate([y_yj, y_hj, y_yi, y_hi], axis=1)),
        })
    return in_maps


def combine(results, pp):
    ns, p, pj = pp["ns"], pp["p"], pp["pj"]
    S1 = 0.0
    S2 = 0.0
    for r in results:
        a = r["o_acc"].astype(np.float64)
        S1 += a[:, 0].sum() + a[:, 1].sum() + a[:, 2].sum()
        S2 += a[:, 3].sum()
    # host: T2_b = sum_j #{i: p_i > p_j}
    h1 = np.bincount(p, minlength=K)
    suf1 = np.concatenate([np.cumsum(h1[::-1])[::-1][1:], [0]])
    S2 += float(suf1[pj].sum())
    c_cnt = S1 - ns
    t_cnt = S2 - ns
    return np.asarray(np.float32(np.float32(c_cnt) / np.float32(t_cnt)))


def kernel(y, y_hat, status, _run_kwargs=None):
    pp = _prep(y, y_hat, status)
    nc = _get_nc(pp["sbh"], pp["sby"])
    in_maps = make_in_maps(pp)
    kw = dict(_run_kwargs or {})
    res = bass_utils.run_bass_kernel_spmd(
        nc, in_maps, core_ids=list(range(NCORES)), **kw)
    out = combine(res.results, pp)
    if _run_kwargs is not None:
        return out, res
    return out


if __name__ == "__main__":
    rng = np.random.default_rng(0)
    y = rng.standard_normal(N).astype(np.float32)
    yh = rng.standard_normal(N).astype(np.float32)
    st = (rng.integers(0, 2, N)).astype(np.int32)
    print(kernel(y, yh, st))
